# revision 1
# baseline (speedup 1.0000x reference)
"""Multi-head causal attention on 8 TRN2 NeuronCores.

Problem: B=4, T=2048, D=1024, H=16 heads of 64. Sharding: core c handles
batch c//2 and head-group c%2 (8 heads = 512 cols of the concat dim). Each
core computes its partial output projection o_g @ Wo_g^T; the host sums the
two partials per batch and adds the bias.

Host prep: x is transposed and cast to bf16 per batch (xT [D, T]), weights
cast to bf16 (wq/wk/wv [D, 512], woT [512, D]).

Per-core kernel (bf16 matmuls, fp32 accumulation):
  QT[hd, t], KT[hd, t]  (hd-major so the K=64 score contraction is direct)
  V[t, hd] with a ones column per head (V_aug)
  per head pair: scoresT[s, q] = K Q^T with the two heads' K=64 matmuls on
  disjoint PE row groups (concurrent); exp fused into the PSUM->SBUF
  activation on ScalarE giving PT[s, q] bf16; causal = multiply the
  diagonal 128-block by an upper-triangular 0/1 mask after exp (scores are
  O(1), no max needed).
  PV: out[q, 0:65|65:130] = P @ V_aug accumulated over s-tiles; col 64 of
  each half is the softmax denominator. Normalize with a per-partition
  reciprocal, DMA-transpose [128,128] into oT[hd, t].
  proj: partial[t, :] = oT^T @ WoT_g (fused into the last pair's PV loop).
"""

import numpy as np
import ml_dtypes
from contextlib import ExitStack

import concourse.mybir as mybir
import concourse.tile as tile
from concourse import bacc
from concourse.bass_utils import run_bass_kernel_spmd

F32 = mybir.dt.float32
BF16 = mybir.dt.bfloat16

B, T, D, H = 4, 2048, 1024, 16
HD = 64          # head dim
HG = 8           # heads per core
GW = HG * HD     # 512, group width
NT = T // 128    # 16 t-tiles
NK = D // 128    # 8 d-tiles
N_CORES = 8

# ragged PT stripe offsets: stripe j holds cols q=128j..2048
_PT_OFF = [2048 * j - 64 * j * (j - 1) for j in range(NT + 1)]
PT_LEN = _PT_OFF[NT]  # 17408


def _build():
    nc = bacc.Bacc("TRN2", target_bir_lowering=False, debug=False,
                   num_devices=N_CORES)
    xT_d = nc.dram_tensor("xT", [D, T], BF16, kind="ExternalInput").ap()
    wq_d = nc.dram_tensor("wq", [D, GW], BF16, kind="ExternalInput").ap()
    wk_d = nc.dram_tensor("wk", [D, GW], BF16, kind="ExternalInput").ap()
    wv_d = nc.dram_tensor("wv", [D, GW], BF16, kind="ExternalInput").ap()
    wo_d = nc.dram_tensor("woT", [GW, D], BF16, kind="ExternalInput").ap()
    tri_d = nc.dram_tensor("tri", [128, 128], BF16, kind="ExternalInput").ap()
    out_d = nc.dram_tensor("out", [T, D], F32, kind="ExternalOutput").ap()

    with tile.TileContext(nc) as tc, ExitStack() as ctx:
        perm = ctx.enter_context(tc.tile_pool(name="perm", bufs=1))
        psA = ctx.enter_context(tc.tile_pool(name="psA", bufs=2, space="PSUM"))
        psB = ctx.enter_context(tc.tile_pool(name="psB", bufs=2, space="PSUM"))
        ps_o = ctx.enter_context(tc.tile_pool(name="ps_o", bufs=2, space="PSUM"))

        tri = perm.tile([128, 128], BF16, tag="tri")
        nc.sync.dma_start(tri[:], tri_d[:])

        qT = perm.tile([128, 4, T], BF16, tag="qT")
        kT = perm.tile([128, 4, T], BF16, tag="kT")
        vsb = perm.tile([128, NT, HG * (HD + 1)], BF16, tag="vsb")
        wob = perm.tile([128, 4, D], BF16, tag="wob")
        oT = perm.tile([128, 4, T], BF16, tag="oT")

        # ones columns for V_aug
        vcols = vsb.rearrange("p j (h c) -> p j h c", c=HD + 1)
        nc.vector.memset(vcols[:, :, :, HD:HD + 1], 1.0)

        def scores_stripe(pair, pts, j):
            """scoresT + fused exp for both heads of one stripe; the two
            K=64 matmuls land on disjoint PE row groups concurrently"""
            m = pair
            if True:
                qa = 128 * j
                first = True
                while qa < T:
                    # chunk up to the next 1024 boundary (<=1024 wide)
                    w = min(1024 - (qa % 1024) if first else 1024, T - qa)
                    first = False
                    pss = []
                    for hh in range(2):
                        base = 64 * hh
                        ps = psA.tile([128, 1024], F32, tag="psA",
                                      name=f"s{pair}_{hh}_{j}_{qa}")
                        a = 0
                        while a < w:
                            b = min(a + 512, w)
                            nc.tensor.matmul(
                                ps[:, a:b],
                                kT[base:base + 64, m, 128 * j:128 * (j + 1)],
                                qT[base:base + 64, m, qa + a:qa + b],
                                start=True, stop=True)
                            a = b
                        pss.append(ps)
                    o0 = _PT_OFF[j] + (qa - 128 * j)
                    for hh in range(2):
                        nc.scalar.activation(
                            pts[hh][:, o0:o0 + w], pss[hh][:, :w],
                            mybir.ActivationFunctionType.Exp, scale=0.125)
                    qa += w
                o0 = _PT_OFF[j]
                for hh in range(2):
                    nc.vector.tensor_mul(pts[hh][:, o0:o0 + 128],
                                         pts[hh][:, o0:o0 + 128], tri[:])

        def pv_i(pair, pts, i, smp, after_i=None):
            """PV + normalize + DMA-transpose into oT for one q-tile"""
            if True:
                po = ps_o.tile([128, 2 * (HD + 1)], F32, tag="po")
                for hh in range(2):
                    h = 2 * pair + hh
                    pt = pts[hh]
                    c0 = (HD + 1) * hh
                    for j in range(i + 1):
                        nc.tensor.matmul(
                            po[:, c0:c0 + HD + 1],
                            pt[:, _PT_OFF[j] + 128 * (i - j):
                               _PT_OFF[j] + 128 * (i - j) + 128],
                            vsb[:, j, (HD + 1) * h:(HD + 1) * (h + 1)],
                            start=(j == 0), stop=(j == i))
                recip = smp.tile([128, 2], F32, tag="recip")
                pov = po.rearrange("p (h c) -> p h c", c=HD + 1)
                nc.vector.reciprocal(recip[:], pov[:, :, HD])
                onat = smp.tile([128, 128], BF16, tag="onat")
                for hh in range(2):
                    c0 = (HD + 1) * hh
                    nc.vector.tensor_scalar_mul(
                        onat[:, 64 * hh:64 * hh + 64],
                        po[:, c0:c0 + HD], recip[:, hh:hh + 1])
                nc.sync.dma_start(oT[:, pair, 128 * i:128 * (i + 1)],
                                  onat[:], transpose=True)
                if after_i is not None:
                    after_i(i)

        with tc.tile_pool(name="ph1", bufs=1) as ph1, \
             tc.tile_pool(name="ptp", bufs=2) as ptp, \
             tc.tile_pool(name="sm", bufs=8) as smp, \
             tc.tile_pool(name="outp", bufs=2) as outp:
            xT = ph1.tile([128, NK, T], BF16, tag="xT")
            wqb = ph1.tile([128, NK, GW], BF16, tag="wqb")
            wkb = ph1.tile([128, NK, GW], BF16, tag="wkb")
            wvb = ph1.tile([128, NK, GW], BF16, tag="wvb")

            qs = [nc.sync, nc.scalar, nc.gpsimd]
            for k in range(NK):
                qs[k % 3].dma_start(xT[:, k, :], xT_d[128 * k:128 * (k + 1), :])
                qs[(k + 1) % 3].dma_start(wqb[:, k, :],
                                          wq_d[128 * k:128 * (k + 1), :])
            for k in range(NK):
                qs[k % 3].dma_start(wkb[:, k, :], wk_d[128 * k:128 * (k + 1), :])
            for k in range(NK):
                qs[(k + 1) % 3].dma_start(wvb[:, k, :],
                                          wv_d[128 * k:128 * (k + 1), :])
            for k in range(4):
                qs[k % 3].dma_start(wob[:, k, :], wo_d[128 * k:128 * (k + 1), :])

            # ---- QKV projections (hd-tile m order, pair-0 slices first) ----
            def qkT_mtile(m):
                for c in range(4):  # t chunks of 512
                    for (wbt, dst) in ((wqb, qT), (wkb, kT)):
                        ps = psB.tile([128, 512], F32, tag="psB")
                        for k in range(NK):
                            nc.tensor.matmul(
                                ps[:], wbt[:, k, 128 * m:128 * (m + 1)],
                                xT[:, k, 512 * c:512 * (c + 1)],
                                start=(k == 0), stop=(k == NK - 1))
                        nc.vector.tensor_copy(
                            dst[:, m, 512 * c:512 * (c + 1)], ps[:])

            def v_jtile(j):
                ps = psB.tile([128, 512], F32, tag="psB")
                for k in range(NK):
                    nc.tensor.matmul(ps[:],
                                     xT[:, k, 128 * j:128 * (j + 1)],
                                     wvb[:, k, :],
                                     start=(k == 0), stop=(k == NK - 1))
                nc.vector.tensor_copy(vcols[:, j, :, :HD], ps[:])

            qkT_mtile(0)

            def proj_i(i):
                ost = outp.tile([128, D], F32, tag="ost", name=f"ost{i}")
                for n in range(2):
                    ps = psB.tile([128, 512], F32, tag="psB")
                    for k in range(4):
                        nc.tensor.matmul(ps[:],
                                         oT[:, k, 128 * i:128 * (i + 1)],
                                         wob[:, k, 512 * n:512 * (n + 1)],
                                         start=(k == 0), stop=(k == 3))
                    nc.vector.tensor_copy(ost[:, 512 * n:512 * (n + 1)],
                                          ps[:])
                nc.scalar.dma_start(out_d[128 * i:128 * (i + 1), :], ost[:])

            # ---- attention head pairs: stripe j scores followed
            # immediately by the i=j PV chain, so PE work interleaves the
            # ACT-bound exp stream stripe-by-stripe ----
            for pair in range(4):
                pts = [ptp.tile([128, PT_LEN], BF16, tag="pt",
                                name=f"pt{pair}_{hh}") for hh in range(2)]
                for j in range(NT):
                    scores_stripe(pair, pts, j)
                    if pair == 0:
                        v_jtile(j)
                    pv_i(pair, pts, j, smp,
                         after_i=proj_i if pair == 3 else None)
                if pair + 1 < 4:
                    qkT_mtile(pair + 1)

    nc.compile()
    return nc


_NC_CACHE = None


def _get_nc():
    global _NC_CACHE
    if _NC_CACHE is None:
        _NC_CACHE = _build()
    return _NC_CACHE


def _prep_in_maps(x, Wq, Wk, Wv, Wo):
    bf = ml_dtypes.bfloat16
    tri = np.triu(np.ones((128, 128), dtype=bf))
    in_maps = []
    for c in range(N_CORES):
        b, g = c // 2, c % 2
        hsl = slice(HG * g, HG * (g + 1))
        in_maps.append({
            "xT": np.ascontiguousarray(x[b].T).astype(bf),
            "wq": np.ascontiguousarray(
                Wq[hsl].transpose(1, 0, 2).reshape(D, GW)).astype(bf),
            "wk": np.ascontiguousarray(
                Wk[hsl].transpose(1, 0, 2).reshape(D, GW)).astype(bf),
            "wv": np.ascontiguousarray(
                Wv[hsl].transpose(1, 0, 2).reshape(D, GW)).astype(bf),
            "woT": np.ascontiguousarray(
                Wo[:, GW * g:GW * (g + 1)].T).astype(bf),
            "tri": tri,
        })
    return in_maps


def kernel(x, Wq, Wk, Wv, Wo, bo, _trace=False, _tmpdir=None):
    nc = _get_nc()
    x = np.asarray(x, dtype=np.float32)
    bo = np.asarray(bo, dtype=np.float32)
    in_maps = _prep_in_maps(x, np.asarray(Wq, np.float32),
                            np.asarray(Wk, np.float32),
                            np.asarray(Wv, np.float32),
                            np.asarray(Wo, np.float32))
    res = run_bass_kernel_spmd(nc, in_maps, core_ids=list(range(N_CORES)),
                               trace=_trace, tmpdir=_tmpdir)
    out = np.empty((B, T, D), dtype=np.float32)
    for b in range(B):
        out[b] = res.results[2 * b]["out"] + res.results[2 * b + 1]["out"] + bo
    if _trace:
        return out, res
    return out



# revision 2
# speedup vs baseline: 1.0670x; 1.0670x over previous
"""Multi-head causal attention on 8 TRN2 NeuronCores.

Problem: B=4, T=2048, D=1024, H=16 heads of 64. Sharding: core c handles
batch c//2 and head-group c%2 (8 heads = 512 cols of the concat dim). Each
core computes its partial output projection o_g @ Wo_g^T; the host sums the
two partials per batch and adds the bias.

Host prep: x is transposed and cast to bf16 per batch (xT [D, T]), weights
cast to bf16 (wq/wk/wv [D, 512], woT [512, D]).

Per-core kernel (bf16 matmuls for QKV/scores/proj, fp8 P*V, fp32 accum):
  QT[hd, t], KT[hd, t]  (hd-major so the K=64 score contraction is direct)
  V[t, hd] in fp8 with a ones column per head (V_aug)
  scores: per stripe j, 512-col chunks; the two heads' K=64 matmuls write
  adjacent PSUM banks of one [128,1024] tile and land on disjoint PE row
  groups (concurrent). Causal mask = additive -1e6 upper-tri into the PSUM
  diagonal chunk pre-exp (one 3D-AP DVE add for both heads). One exp call
  per chunk covers both heads via 3D APs, writing fp8 PT stripes.
  PV: out[q, 0:65|65:130] = P @ V_aug (fp8) accumulated over s-tiles; col
  64 of each half is the softmax denominator. Normalize with a
  per-partition reciprocal, DMA-transpose [128,128] into oT[hd, t].
  proj: partial[t, :] = oT^T @ WoT_g interleaved into pair 3's stripes.
  ~110 warmup matmuls at start keep the PE HAM clock-gate warm through
  the input-DMA phase; qkT units of pair p+1 fill pair p's ACT-bound gaps.
"""

import numpy as np
import ml_dtypes
from contextlib import ExitStack

import concourse.mybir as mybir
import concourse.tile as tile
from concourse import bacc
from concourse.bass_utils import run_bass_kernel_spmd

F32 = mybir.dt.float32
BF16 = mybir.dt.bfloat16
FP8 = mybir.dt.float8e4

B, T, D, H = 4, 2048, 1024, 16
HD = 64          # head dim
HG = 8           # heads per core
GW = HG * HD     # 512, group width
NT = T // 128    # 16 t-tiles
NK = D // 128    # 8 d-tiles
N_CORES = 8
N_WARMUP = 110   # PE warmup matmuls to hold HAM at K=8/8 through DMA phase

# ragged PT stripe offsets: stripe j holds cols q=128j..2048
_PT_OFF = [2048 * j - 64 * j * (j - 1) for j in range(NT + 1)]
PT_LEN = _PT_OFF[NT]  # 17408


def _build():
    nc = bacc.Bacc("TRN2", target_bir_lowering=False, debug=False,
                   num_devices=N_CORES)
    xT_d = nc.dram_tensor("xT", [D, T], BF16, kind="ExternalInput").ap()
    wq_d = nc.dram_tensor("wq", [D, GW], BF16, kind="ExternalInput").ap()
    wk_d = nc.dram_tensor("wk", [D, GW], BF16, kind="ExternalInput").ap()
    wv_d = nc.dram_tensor("wv", [D, GW], BF16, kind="ExternalInput").ap()
    wo_d = nc.dram_tensor("woT", [GW, D], BF16, kind="ExternalInput").ap()
    tri_d = nc.dram_tensor("tri2", [128, 256], F32, kind="ExternalInput").ap()
    out_d = nc.dram_tensor("out", [T, D], F32, kind="ExternalOutput").ap()

    with tile.TileContext(nc) as tc, ExitStack() as ctx:
        perm = ctx.enter_context(tc.tile_pool(name="perm", bufs=1))
        psA = ctx.enter_context(tc.tile_pool(name="psA", bufs=2, space="PSUM"))
        psB = ctx.enter_context(tc.tile_pool(name="psB", bufs=2, space="PSUM"))
        ps_o = ctx.enter_context(tc.tile_pool(name="ps_o", bufs=2, space="PSUM"))

        tri2 = perm.tile([128, 256], F32, tag="tri2")
        nc.sync.dma_start(tri2[:], tri_d[:])
        tri3 = tri2.rearrange("p (h w) -> p h w", h=2)

        qT = perm.tile([128, 4, T], BF16, tag="qT")
        kT = perm.tile([128, 4, T], BF16, tag="kT")
        vsb = perm.tile([128, NT, HG * (HD + 1)], FP8, tag="vsb")
        wob = perm.tile([128, 4, D], BF16, tag="wob")
        oT = perm.tile([128, 4, T], BF16, tag="oT")
        wsrc = perm.tile([128, 128], BF16, tag="wsrc")

        nc.vector.memset(wsrc[:], 0.0)
        # ones columns for V_aug
        vcols = vsb.rearrange("p j (h c) -> p j h c", c=HD + 1)
        nc.vector.memset(vcols[:, :, :, HD:HD + 1], 1.0)

        def scores_stripe(pair, pt3, j):
            """scoresT chunks for both heads of one stripe; each 512-chunk:
            head A -> bank0 cols, head B -> bank1 cols of one psA tile so
            the two K=64 matmuls run on disjoint PE row groups; additive
            causal mask on the diagonal chunk; one fused exp per chunk."""
            m = pair
            wj = T - 128 * j
            nch = (wj + 511) // 512
            for c in range(nch):
                w = min(512, wj - 512 * c)
                qa = 128 * j + 512 * c
                ps = psA.tile([128, 1024], F32, tag="psA",
                              name=f"s{pair}_{j}_{c}")
                for hh in range(2):
                    base = 64 * hh
                    nc.tensor.matmul(
                        ps[:, 512 * hh:512 * hh + w],
                        kT[base:base + 64, m, 128 * j:128 * (j + 1)],
                        qT[base:base + 64, m, qa:qa + w],
                        start=True, stop=True)
                ps3 = ps.rearrange("p (h w) -> p h w", h=2)
                if c == 0:
                    # causal: -1e6 above the diagonal of the diag block
                    nc.vector.tensor_add(ps3[:, :, 0:128], ps3[:, :, 0:128],
                                         tri3[:])
                o0 = _PT_OFF[j] + 512 * c
                nc.scalar.activation(
                    pt3[:, :, o0:o0 + w], ps3[:, :, :w],
                    mybir.ActivationFunctionType.Exp, scale=0.125)

        def pv_i(pair, pt3, i, smp, after_i=None):
            """PV + normalize + DMA-transpose into oT for one q-tile"""
            po = ps_o.tile([128, 2 * (HD + 1)], F32, tag="po")
            for hh in range(2):
                h = 2 * pair + hh
                c0 = (HD + 1) * hh
                for j in range(i + 1):
                    nc.tensor.matmul(
                        po[:, c0:c0 + HD + 1],
                        pt3[:, hh, _PT_OFF[j] + 128 * (i - j):
                            _PT_OFF[j] + 128 * (i - j) + 128],
                        vsb[:, j, (HD + 1) * h:(HD + 1) * (h + 1)],
                        start=(j == 0), stop=(j == i))
            recip = smp.tile([128, 2], F32, tag="recip")
            pov = po.rearrange("p (h c) -> p h c", c=HD + 1)
            nc.vector.reciprocal(recip[:], pov[:, :, HD])
            onat = smp.tile([128, 128], BF16, tag="onat")
            for hh in range(2):
                c0 = (HD + 1) * hh
                nc.vector.tensor_scalar_mul(
                    onat[:, 64 * hh:64 * hh + 64],
                    po[:, c0:c0 + HD], recip[:, hh:hh + 1])
            nc.sync.dma_start(oT[:, pair, 128 * i:128 * (i + 1)],
                              onat[:], transpose=True)
            if after_i is not None:
                after_i(i)

        with tc.tile_pool(name="ph1", bufs=1) as ph1, \
             tc.tile_pool(name="ptp", bufs=2) as ptp, \
             tc.tile_pool(name="sm", bufs=8) as smp, \
             tc.tile_pool(name="outp", bufs=2) as outp:
            xT = ph1.tile([128, NK, T], BF16, tag="xT")
            wqb = ph1.tile([128, NK, GW], BF16, tag="wqb")
            wkb = ph1.tile([128, NK, GW], BF16, tag="wkb")
            wvb = ph1.tile([128, NK, GW], BF16, tag="wvb")

            # PE warmup: back-to-back tiny matmuls with no input deps keep
            # the HAM clock-gate at K=8/8 while input DMAs stream in
            for wi in range(N_WARMUP):
                pw = psB.tile([128, 512], F32, tag="psB", name=f"warm{wi}")
                nc.tensor.matmul(pw[:, 0:128], wsrc[:], wsrc[:],
                                 start=True, stop=True)

            qs = [nc.sync, nc.scalar, nc.gpsimd]
            for k in range(NK):
                qs[k % 3].dma_start(xT[:, k, :], xT_d[128 * k:128 * (k + 1), :])
                qs[(k + 1) % 3].dma_start(wqb[:, k, :],
                                          wq_d[128 * k:128 * (k + 1), :])
                qs[(k + 2) % 3].dma_start(wkb[:, k, :],
                                          wk_d[128 * k:128 * (k + 1), :])
            for k in range(NK):
                qs[k % 3].dma_start(wvb[:, k, :], wv_d[128 * k:128 * (k + 1), :])
            for k in range(4):
                qs[(k + 1) % 3].dma_start(wob[:, k, :],
                                          wo_d[128 * k:128 * (k + 1), :])

            # ---- QKV projection units (8 per m-tile) ----
            def qkT_unit(m, u):
                c, qk = u // 2, u % 2
                wbt, dst = ((wqb, qT), (wkb, kT))[qk]
                ps = psB.tile([128, 512], F32, tag="psB")
                for k in range(NK):
                    nc.tensor.matmul(
                        ps[:], wbt[:, k, 128 * m:128 * (m + 1)],
                        xT[:, k, 512 * c:512 * (c + 1)],
                        start=(k == 0), stop=(k == NK - 1))
                nc.vector.tensor_copy(
                    dst[:, m, 512 * c:512 * (c + 1)], ps[:])

            def v_jtile(j):
                ps = psB.tile([128, 512], F32, tag="psB")
                for k in range(NK):
                    nc.tensor.matmul(ps[:],
                                     xT[:, k, 128 * j:128 * (j + 1)],
                                     wvb[:, k, :],
                                     start=(k == 0), stop=(k == NK - 1))
                nc.vector.tensor_copy(vcols[:, j, :, :HD], ps[:])

            for u in range(8):
                qkT_unit(0, u)

            def proj_i(i):
                ost = outp.tile([128, D], F32, tag="ost", name=f"ost{i}")
                for n in range(2):
                    ps = psB.tile([128, 512], F32, tag="psB")
                    for k in range(4):
                        nc.tensor.matmul(ps[:],
                                         oT[:, k, 128 * i:128 * (i + 1)],
                                         wob[:, k, 512 * n:512 * (n + 1)],
                                         start=(k == 0), stop=(k == 3))
                    nc.vector.tensor_copy(ost[:, 512 * n:512 * (n + 1)],
                                          ps[:])
                nc.gpsimd.dma_start(out_d[128 * i:128 * (i + 1), :], ost[:])

            # ---- attention head pairs: stripe j scores then the i=j PV
            # chain; qkT units of the next pair fill ACT-bound gaps ----
            for pair in range(4):
                pt = ptp.tile([128, 2, PT_LEN], FP8, tag="pt",
                              name=f"pt{pair}")
                for j in range(NT):
                    scores_stripe(pair, pt, j)
                    if pair == 0:
                        v_jtile(j)
                    pv_i(pair, pt, j, smp,
                         after_i=proj_i if pair == 3 else None)
                    if pair < 3 and j % 2 == 1:
                        qkT_unit(pair + 1, j // 2)

    nc.compile()
    return nc


_NC_CACHE = None


def _get_nc():
    global _NC_CACHE
    if _NC_CACHE is None:
        _NC_CACHE = _build()
    return _NC_CACHE


def _prep_in_maps(x, Wq, Wk, Wv, Wo):
    bf = ml_dtypes.bfloat16
    tri = np.where(np.triu(np.ones((128, 128), dtype=bool)),
                   np.float32(0.0), np.float32(-1e6))
    tri2 = np.concatenate([tri, tri], axis=1).astype(np.float32)
    in_maps = []
    for c in range(N_CORES):
        b, g = c // 2, c % 2
        hsl = slice(HG * g, HG * (g + 1))
        in_maps.append({
            "xT": np.ascontiguousarray(x[b].T).astype(bf),
            "wq": np.ascontiguousarray(
                Wq[hsl].transpose(1, 0, 2).reshape(D, GW)).astype(bf),
            "wk": np.ascontiguousarray(
                Wk[hsl].transpose(1, 0, 2).reshape(D, GW)).astype(bf),
            "wv": np.ascontiguousarray(
                Wv[hsl].transpose(1, 0, 2).reshape(D, GW)).astype(bf),
            "woT": np.ascontiguousarray(
                Wo[:, GW * g:GW * (g + 1)].T).astype(bf),
            "tri2": tri2,
        })
    return in_maps


def kernel(x, Wq, Wk, Wv, Wo, bo, _trace=False, _tmpdir=None):
    nc = _get_nc()
    x = np.asarray(x, dtype=np.float32)
    bo = np.asarray(bo, dtype=np.float32)
    in_maps = _prep_in_maps(x, np.asarray(Wq, np.float32),
                            np.asarray(Wk, np.float32),
                            np.asarray(Wv, np.float32),
                            np.asarray(Wo, np.float32))
    res = run_bass_kernel_spmd(nc, in_maps, core_ids=list(range(N_CORES)),
                               trace=_trace, tmpdir=_tmpdir)
    out = np.empty((B, T, D), dtype=np.float32)
    for b in range(B):
        out[b] = res.results[2 * b]["out"] + res.results[2 * b + 1]["out"] + bo
    if _trace:
        return out, res
    return out
